# revision 11
# baseline (speedup 1.0000x reference)
"""Trainium2 Bass kernel for nn_CLinear (group-quantized linear layer).

Computes out = x @ dequant(qweight).T + bias where
  x:       [4, 2048, 4096] f32
  qweight: [11008, 16, 256] int8 (group-quantized, G=256)
  scale:   [11008, 16, 1]   f32  (w = qweight / scale)
  bias:    [11008]          f32
  out:     [4, 2048, 11008] f32

Sharding: column-parallel (tensor-parallel over out_features) across 8
NeuronCores.  11008 = 8 * 1376 exactly, so every core owns a contiguous
1376-column output shard (the matmul free dim needs no 128-alignment).

Per-core kernel structure:
  - x is pre-transposed on the host into K-permuted bf16 lhsT tiles
    xt[(m p), (u tc)] = x[128m + tc, 1024*(p//32) + 32u + (p%32)]: one
    contiguous full-partition 1 MB DMA per token tile, no on-chip
    transpose work.
  - The weight shard is host-dequantized to bf16 in the same K-permuted
    layout and stored chunk-major in DRAM (32 fully contiguous 352 KB
    single-k-tile chunks) so the weight stream reads HBM sequentially.
    Chunks ride the three DGE queues (scalar/gpsimd/sync).
  - Warm-up: ~10 us of throwaway matmuls keep the PE busy from t~2us so
    the HAM clock gate is at 8/8 before the first real matmul, then the
    first TWO token tiles are computed chunk-major (m0/m1 interleaved per
    weight chunk, all 6 PSUM banks accumulating) so the PE does ~37 us of
    useful work while the weight stream lands.
  - Steady state: per token tile, 3 out-blocks x 32 accumulating bf16
    matmuls into PSUM f32; DVE adds bias during PSUM->SBUF evict; the f32
    result DMAs out on the scalar queue.
"""

import numpy as np

import concourse.bass as bass
import concourse.mybir as mybir
import concourse.tile as tile
from concourse import bacc
from concourse.bass_utils import run_bass_kernel_spmd

P = 128
B, S, IN, OUT, G = 4, 2048, 4096, 11008, 256
NCORES = 8
T = B * S                      # 8192 tokens
OUT_SH = OUT // NCORES         # 1376 out features per core (exact)
NG = IN // G                   # 16 quant groups per row
F32 = mybir.dt.float32
BF16 = mybir.dt.bfloat16

UCH = 1                        # k-tiles per weight chunk
NWARM = 40                     # PE warm-up dummy matmuls
WARM_TILES = 2                 # token tiles computed chunk-major at start


def _n_blocks(out_sh, nmax=512):
    blocks = []
    o = 0
    while o < out_sh:
        sz = min(nmax, out_sh - o)
        blocks.append((o, sz))
        o += sz
    return blocks


def emit_kernel(tc, nc, xt_d, wt_d, bb_d, y_d, t_dim, in_dim, out_sh):
    """Emit the per-core kernel IR.

    xt_d: [t_dim, in_dim]           bf16  (K-permuted pre-transposed x)
    wt_d: [kt//UCH, P, UCH, out_sh] bf16  (chunk-major K-permuted weights)
    bb_d: [P, out_sh]               bf16  (bias broadcast across partitions)
    y_d:  [t_dim, out_sh]           f32   (output shard)
    """
    kt = in_dim // P           # k-tiles (u index)
    mt = t_dim // P            # token tiles
    nchunk = kt // UCH
    nblk = _n_blocks(out_sh)

    from contextlib import ExitStack
    ctx = ExitStack()
    const = ctx.enter_context(tc.tile_pool(name="const", bufs=1))
    wtp = ctx.enter_context(tc.tile_pool(name="wt", bufs=1))
    ytp = ctx.enter_context(tc.tile_pool(name="yt", bufs=5))
    outp = ctx.enter_context(tc.tile_pool(name="out", bufs=6))
    psp = ctx.enter_context(tc.tile_pool(name="psum", bufs=2, space="PSUM"))
    wup = ctx.enter_context(tc.tile_pool(name="wup", bufs=1, space="PSUM"))

    # Priority bands (lower = scheduled earlier among ready work):
    PRI_WARM = 1100000  # dummy matmul warm-up, ahead of everything
    PRI_X0 = 1000000    # first two x tiles: needed before any matmul
    PRI_WT = 900000     # weight chunk stream the warm-up tiles pace
    PRI_X = 500000      # steady-state x prefetch: behind the weight
                        # stream, ahead of normal work (evicts/stores)

    # --- PE warm-up: keep the PE busy from ~2us so the HAM clock gate
    # reaches 8/8 before the first real matmul (and never re-throttles).
    with tc.high_priority(offset=PRI_WARM):
        dummy = const.tile([P, 256], BF16)
        nc.vector.memzero(dummy[:])
        wps = wup.tile([P, 256], F32)
        for _ in range(NWARM):
            nc.tensor.matmul(wps[:], dummy[:, :P], dummy[:],
                             start=True, stop=True)

    def load_yt(m, pri, eng=nc.sync, halves=False):
        with tc.high_priority(offset=pri):
            yt = ytp.tile([P, kt, P], BF16, name="yt")
            row = xt_d[m * P:(m + 1) * P, :]
            if halves:   # first half only; caller DMAs the rest later
                eng.dma_start(
                    yt[:, :kt // 2, :].rearrange("p u tc -> p (u tc)"),
                    row[:, :kt // 2 * P],
                )
            else:
                eng.dma_start(yt.rearrange("p u tc -> p (u tc)"), row)
            return yt

    def load_yt_rest(m, yt, eng):
        eng.dma_start(
            yt[:, kt // 2:, :].rearrange("p u tc -> p (u tc)"),
            xt_d[m * P:(m + 1) * P, kt // 2 * P:],
        )

    # First halves of the warm-up x tiles (all the warm-up needs until
    # k-tile 16) load in parallel on sync/gpsimd ahead of the weight
    # stream; the second halves ride mid-stream below.
    yts = {0: load_yt(0, PRI_X0, eng=nc.sync, halves=True),
           1: load_yt(1, PRI_X0, eng=nc.gpsimd, halves=True)}
    with tc.high_priority(offset=PRI_X0):
        biasb = const.tile([P, out_sh], BF16)
        nc.sync.dma_start(biasb[:], bb_d[:, :])

    # --- Weight chunk stream: early chunks only on scalar/gpsimd (sync
    # is busy with yt0 + bias), sync joins for the later chunks, and the
    # warm-up tiles' second halves slot in before k-tile 16's chunk.
    def chunk_queue(g):
        if g < 8:
            return [nc.scalar, nc.gpsimd][g % 2]
        return [nc.sync, nc.scalar, nc.gpsimd][g % 3]

    wts = []
    with tc.high_priority(offset=PRI_WT):
        for g in range(nchunk):
            if g == kt // 2:
                load_yt_rest(0, yts[0], nc.sync)
                load_yt_rest(1, yts[1], nc.gpsimd)
            wtt = wtp.tile([P, UCH, out_sh], BF16, name=f"wt{g}")
            chunk_queue(g).dma_start(wtt[:], wt_d[g])
            wts.append(wtt)

    for m in range(2, min(5, mt)):
        yts[m] = load_yt(m, PRI_X)

    pending = []   # psums awaiting evict, evicted one tile late so the
                   # DVE never blocks the PE's critical path.

    def evict(m, nb, n0, sz, ps):
        t0 = m * P
        ot = outp.tile([P, 512], F32, name="ot")
        nc.vector.tensor_tensor(
            ot[:, :sz], ps, biasb[:, n0:n0 + sz], mybir.AluOpType.add
        )
        nc.scalar.dma_start(y_d[t0:t0 + P, n0:n0 + sz], ot[:, :sz])

    # --- Warm-up tiles m=0..WARM_TILES-1, chunk-major: both tiles'
    # accumulation groups stay open across the whole k loop (6 PSUM
    # banks) and consume each weight chunk as it lands.
    wt_ps = {}
    for m in range(WARM_TILES):
        for nb, (n0, sz) in enumerate(nblk):
            wt_ps[(m, nb)] = psp.tile([P, 512], F32, name=f"ps{nb}")[:, :sz]
    for g in range(nchunk):
        for m in range(WARM_TILES):
            for uu in range(UCH):
                u = g * UCH + uu
                for nb, (n0, sz) in enumerate(nblk):
                    nc.tensor.matmul(
                        wt_ps[(m, nb)],
                        yts[m][:, u, :],
                        wts[g][:, uu, n0:n0 + sz],
                        start=(u == 0),
                        stop=(u == kt - 1),
                        skip_group_check=True,
                    )
    for m in range(WARM_TILES):
        for nb, (n0, sz) in enumerate(nblk):
            pending.append((m, nb, n0, sz, wt_ps[(m, nb)]))
        yts.pop(m)

    # --- Steady state.
    for m in range(WARM_TILES, mt):
        if m + 3 < mt and (m + 3) not in yts:
            yts[m + 3] = load_yt(m + 3, PRI_X)
        for args in pending:
            evict(*args)
        pending = []
        ytf = yts.pop(m)
        for nb, (n0, sz) in enumerate(nblk):
            ps = psp.tile([P, 512], F32, name=f"ps{nb}")[:, :sz]
            for u in range(kt):
                nc.tensor.matmul(
                    ps,
                    ytf[:, u, :],  # [P, 128] contiguous: tokens t0..t0+127
                    wts[u // UCH][:, u % UCH, n0:n0 + sz],
                    start=(u == 0),
                    stop=(u == kt - 1),
                )
            pending.append((m, nb, n0, sz, ps))
    for args in pending:
        evict(*args)

    ctx.close()


def build_nc(t_dim=T, in_dim=IN, out_sh=OUT_SH, debug=False):
    kt = in_dim // P
    nc = bacc.Bacc(
        "TRN2",
        target_bir_lowering=False,
        debug=debug,
        num_devices=NCORES,
        enable_asserts=debug,
    )
    xt_d = nc.dram_tensor("xt", [t_dim, in_dim], BF16, kind="ExternalInput").ap()
    wt_d = nc.dram_tensor("wt", [kt // UCH, P, UCH, out_sh], BF16,
                          kind="ExternalInput").ap()
    bb_d = nc.dram_tensor("biasb", [P, out_sh], BF16, kind="ExternalInput").ap()
    y_d = nc.dram_tensor("y", [t_dim, out_sh], F32, kind="ExternalOutput").ap()
    with tile.TileContext(nc) as tc:
        emit_kernel(tc, nc, xt_d, wt_d, bb_d, y_d, t_dim, in_dim, out_sh)
    nc.compile()
    return nc


_NC_CACHE = {}


def _get_nc():
    if "nc" not in _NC_CACHE:
        _NC_CACHE["nc"] = build_nc()
    return _NC_CACHE["nc"]


def _permute_kt(arr):
    """[rows, IN] -> K-permuted transposed [P, kt, rows]:
    out[32q+r, u, o] = arr[o, 1024q + 32u + r]."""
    rows = arr.shape[0]
    kt = IN // P
    a = arr.reshape(rows, 4, kt, 32)                # [o, q, u, r]
    a = a.transpose(1, 3, 2, 0)                     # [q, r, u, o]
    return np.ascontiguousarray(a.reshape(P, kt, rows))


def prep_inputs(x, qweight, scale, bias):
    """Host-side shard prep. Returns in_maps for run_bass_kernel_spmd."""
    import ml_dtypes
    BF = ml_dtypes.bfloat16
    x = np.asarray(x)
    qw = np.asarray(qweight).reshape(OUT, NG, G)
    sc = np.asarray(scale, dtype=np.float32)
    b = np.asarray(bias, dtype=np.float32)

    # Pre-transposed K-permuted bf16 activations:
    # xt[(m p), (u tc)] = x[128m + tc, 1024*(p//32) + 32u + (p%32)]
    xb = x.reshape(T, IN).astype(BF)
    xt = xb.reshape(T // P, P, 4, IN // P, 32)      # [m, tc, q, u, r]
    xt = xt.transpose(0, 2, 4, 3, 1)                # [m, q, r, u, tc]
    xt = np.ascontiguousarray(xt.reshape(T, IN))

    # Dequantize exactly as the reference does (q / scale, f32), then bf16.
    w = (qw.astype(np.float32) / sc.reshape(OUT, NG, 1)).reshape(OUT, IN)
    w_u16 = w.astype(BF).view(np.uint16)
    bb = b.astype(BF)

    in_maps = []
    for c in range(NCORES):
        sl = slice(c * OUT_SH, (c + 1) * OUT_SH)
        wt = _permute_kt(w_u16[sl]).view(BF)        # [P, kt, OUT_SH]
        # chunk-major: [kt//UCH, P, UCH, OUT_SH], contiguous per chunk
        wtc = np.ascontiguousarray(
            wt.reshape(P, IN // P // UCH, UCH, OUT_SH).transpose(1, 0, 2, 3)
        )
        in_maps.append({
            "xt": xt,
            "wt": wtc,
            "biasb": np.ascontiguousarray(
                np.broadcast_to(bb[sl][None, :], (P, OUT_SH))
            ),
        })
    return in_maps


def run(x, qweight, scale, bias, trace=False):
    nc = _get_nc()
    in_maps = prep_inputs(x, qweight, scale, bias)
    res = run_bass_kernel_spmd(nc, in_maps, core_ids=list(range(NCORES)),
                               trace=trace)
    ys = [np.asarray(res.results[c]["y"]) for c in range(NCORES)]
    out = np.concatenate(ys, axis=1)
    return out.reshape(B, S, OUT).astype(np.float32, copy=False), res


def kernel(x, qweight, scale, bias):
    out, _ = run(x, qweight, scale, bias, trace=False)
    return out


# revision 12
# speedup vs baseline: 1.1892x; 1.1892x over previous
"""Trainium2 Bass kernel for nn_CLinear (group-quantized linear layer).

Computes out = x @ dequant(qweight).T + bias where
  x:       [4, 2048, 4096] f32
  qweight: [11008, 16, 256] int8 (group-quantized, G=256)
  scale:   [11008, 16, 1]   f32  (w = qweight / scale)
  bias:    [11008]          f32
  out:     [4, 2048, 11008] f32

Sharding: column-parallel (tensor-parallel over out_features) across 8
NeuronCores.  11008 = 8 * 1376 exactly, so every core owns a contiguous
1376-column output shard (the matmul free dim needs no 128-alignment).

Per-core kernel structure:
  - x is pre-transposed on the host into K-permuted bf16 lhsT tiles
    xt[(m p), (u tc)] = x[128m + tc, 1024*(p//32) + 32u + (p%32)]: one
    contiguous full-partition 1 MB DMA per token tile, no on-chip
    transpose work.
  - The weight shard is host-dequantized to bf16 in the same K-permuted
    layout and stored chunk-major in DRAM (32 fully contiguous 352 KB
    single-k-tile chunks) so the weight stream reads HBM sequentially.
    Chunks ride the three DGE queues (scalar/gpsimd/sync).
  - Warm-up: ~10 us of throwaway matmuls keep the PE busy from t~2us so
    the HAM clock gate is at 8/8 before the first real matmul, then the
    first TWO token tiles are computed chunk-major (m0/m1 interleaved per
    weight chunk, all 6 PSUM banks accumulating) so the PE does ~37 us of
    useful work while the weight stream lands.
  - Steady state: per token tile, 3 out-blocks x 32 accumulating bf16
    matmuls into PSUM f32; DVE adds bias during PSUM->SBUF evict; the f32
    result DMAs out on the scalar queue.
"""

import numpy as np

import concourse.bass as bass
import concourse.mybir as mybir
import concourse.tile as tile
from concourse import bacc
from concourse.bass_utils import run_bass_kernel_spmd

P = 128
B, S, IN, OUT, G = 4, 2048, 4096, 11008, 256
NCORES = 8
T = B * S                      # 8192 tokens
OUT_SH = OUT // NCORES         # 1376 out features per core (exact)
NG = IN // G                   # 16 quant groups per row
F32 = mybir.dt.float32
BF16 = mybir.dt.bfloat16

UCH = 1                        # k-tiles per weight chunk
NWARM = 80                     # PE warm-up dummy matmuls (~9.5 us bridge
                               # from engine start to the first weight chunk)
WARM_TILES = 2                 # token tiles computed chunk-major at start


def _n_blocks(out_sh, nmax=512):
    blocks = []
    o = 0
    while o < out_sh:
        sz = min(nmax, out_sh - o)
        blocks.append((o, sz))
        o += sz
    return blocks


def emit_kernel(tc, nc, xt_d, wt_d, bb_d, y_d, t_dim, in_dim, out_sh):
    """Emit the per-core kernel IR.

    xt_d: [t_dim, in_dim]           bf16  (K-permuted pre-transposed x)
    wt_d: [kt//UCH, P, UCH, out_sh] bf16  (chunk-major K-permuted weights)
    bb_d: [P, out_sh]               bf16  (bias broadcast across partitions)
    y_d:  [t_dim, out_sh]           f32   (output shard)
    """
    kt = in_dim // P           # k-tiles (u index)
    mt = t_dim // P            # token tiles
    nchunk = kt // UCH
    nblk = _n_blocks(out_sh)

    from contextlib import ExitStack
    ctx = ExitStack()
    const = ctx.enter_context(tc.tile_pool(name="const", bufs=1))
    wtp = ctx.enter_context(tc.tile_pool(name="wt", bufs=1))
    ytp = ctx.enter_context(tc.tile_pool(name="yt", bufs=5))
    outp = ctx.enter_context(tc.tile_pool(name="out", bufs=6))
    psp = ctx.enter_context(tc.tile_pool(name="psum", bufs=2, space="PSUM"))
    wup = ctx.enter_context(tc.tile_pool(name="wup", bufs=1, space="PSUM"))

    # Priority bands (lower = scheduled earlier among ready work):
    PRI_WARM = 1100000  # dummy matmul warm-up, ahead of everything
    PRI_X0 = 1000000    # first two x tiles: needed before any matmul
    PRI_WT = 900000     # weight chunk stream the warm-up tiles pace
    PRI_X = 500000      # steady-state x prefetch: behind the weight
                        # stream, ahead of normal work (evicts/stores)

    # --- PE warm-up: keep the PE busy from ~2us so the HAM clock gate
    # reaches 8/8 before the first real matmul (and never re-throttles).
    with tc.high_priority(offset=PRI_WARM):
        dummy = const.tile([P, 256], BF16)
        nc.vector.memzero(dummy[:])
        wps = wup.tile([P, 256], F32)
        for _ in range(NWARM):
            nc.tensor.matmul(wps[:], dummy[:, :P], dummy[:],
                             start=True, stop=True)

    def load_yt(m, pri, eng=nc.sync, halves=False):
        with tc.high_priority(offset=pri):
            yt = ytp.tile([P, kt, P], BF16, name="yt")
            row = xt_d[m * P:(m + 1) * P, :]
            if halves:   # first half only; caller DMAs the rest later
                eng.dma_start(
                    yt[:, :kt // 2, :].rearrange("p u tc -> p (u tc)"),
                    row[:, :kt // 2 * P],
                )
            else:
                eng.dma_start(yt.rearrange("p u tc -> p (u tc)"), row)
            return yt

    def load_yt_rest(m, yt, eng):
        eng.dma_start(
            yt[:, kt // 2:, :].rearrange("p u tc -> p (u tc)"),
            xt_d[m * P:(m + 1) * P, kt // 2 * P:],
        )

    # First halves of the warm-up x tiles (all the warm-up needs until
    # k-tile 16) load in parallel on sync/gpsimd ahead of the weight
    # stream; the second halves ride mid-stream below.
    yts = {0: load_yt(0, PRI_X0, eng=nc.sync, halves=True),
           1: load_yt(1, PRI_X0, eng=nc.gpsimd, halves=True)}
    with tc.high_priority(offset=PRI_X0):
        biasb = const.tile([P, out_sh], BF16)
        nc.sync.dma_start(biasb[:], bb_d[:, :])

    # --- Weight chunk stream: early chunks only on scalar/gpsimd (sync
    # is busy with yt0 + bias), sync joins for the later chunks, and the
    # warm-up tiles' second halves slot in before k-tile 16's chunk.
    def chunk_queue(g):
        if g < 8:
            return [nc.scalar, nc.gpsimd][g % 2]
        return [nc.sync, nc.scalar, nc.gpsimd][g % 3]

    wts = []
    with tc.high_priority(offset=PRI_WT):
        for g in range(nchunk):
            if g == kt // 2:
                load_yt_rest(0, yts[0], nc.sync)
                load_yt_rest(1, yts[1], nc.gpsimd)
            wtt = wtp.tile([P, UCH, out_sh], BF16, name=f"wt{g}")
            chunk_queue(g).dma_start(wtt[:], wt_d[g])
            wts.append(wtt)

    for m in range(2, min(5, mt)):
        yts[m] = load_yt(m, PRI_X)

    pending = []   # psums awaiting evict, evicted one tile late so the
                   # DVE never blocks the PE's critical path.

    def evict(m, nb, n0, sz, ps):
        t0 = m * P
        ot = outp.tile([P, 512], F32, name="ot")
        nc.vector.tensor_tensor(
            ot[:, :sz], ps, biasb[:, n0:n0 + sz], mybir.AluOpType.add
        )
        nc.scalar.dma_start(y_d[t0:t0 + P, n0:n0 + sz], ot[:, :sz])

    # --- Warm-up tiles m=0..WARM_TILES-1, chunk-major: both tiles'
    # accumulation groups stay open across the whole k loop (6 PSUM
    # banks) and consume each weight chunk as it lands.
    wt_ps = {}
    for m in range(WARM_TILES):
        for nb, (n0, sz) in enumerate(nblk):
            wt_ps[(m, nb)] = psp.tile([P, 512], F32, name=f"ps{nb}")[:, :sz]
    for g in range(nchunk):
        for m in range(WARM_TILES):
            for uu in range(UCH):
                u = g * UCH + uu
                for nb, (n0, sz) in enumerate(nblk):
                    nc.tensor.matmul(
                        wt_ps[(m, nb)],
                        yts[m][:, u, :],
                        wts[g][:, uu, n0:n0 + sz],
                        start=(u == 0),
                        stop=(u == kt - 1),
                        skip_group_check=True,
                    )
    for m in range(WARM_TILES):
        for nb, (n0, sz) in enumerate(nblk):
            pending.append((m, nb, n0, sz, wt_ps[(m, nb)]))
        yts.pop(m)

    # --- Steady state.
    for m in range(WARM_TILES, mt):
        if m + 3 < mt and (m + 3) not in yts:
            yts[m + 3] = load_yt(m + 3, PRI_X)
        for args in pending:
            evict(*args)
        pending = []
        ytf = yts.pop(m)
        for nb, (n0, sz) in enumerate(nblk):
            ps = psp.tile([P, 512], F32, name=f"ps{nb}")[:, :sz]
            for u in range(kt):
                nc.tensor.matmul(
                    ps,
                    ytf[:, u, :],  # [P, 128] contiguous: tokens t0..t0+127
                    wts[u // UCH][:, u % UCH, n0:n0 + sz],
                    start=(u == 0),
                    stop=(u == kt - 1),
                )
            pending.append((m, nb, n0, sz, ps))
    for args in pending:
        evict(*args)

    ctx.close()


def build_nc(t_dim=T, in_dim=IN, out_sh=OUT_SH, debug=False):
    kt = in_dim // P
    nc = bacc.Bacc(
        "TRN2",
        target_bir_lowering=False,
        debug=debug,
        num_devices=NCORES,
        enable_asserts=debug,
    )
    xt_d = nc.dram_tensor("xt", [t_dim, in_dim], BF16, kind="ExternalInput").ap()
    wt_d = nc.dram_tensor("wt", [kt // UCH, P, UCH, out_sh], BF16,
                          kind="ExternalInput").ap()
    bb_d = nc.dram_tensor("biasb", [P, out_sh], BF16, kind="ExternalInput").ap()
    y_d = nc.dram_tensor("y", [t_dim, out_sh], F32, kind="ExternalOutput").ap()
    with tile.TileContext(nc) as tc:
        emit_kernel(tc, nc, xt_d, wt_d, bb_d, y_d, t_dim, in_dim, out_sh)
    nc.compile()
    return nc


_NC_CACHE = {}


def _get_nc():
    if "nc" not in _NC_CACHE:
        _NC_CACHE["nc"] = build_nc()
    return _NC_CACHE["nc"]


def _permute_kt(arr):
    """[rows, IN] -> K-permuted transposed [P, kt, rows]:
    out[32q+r, u, o] = arr[o, 1024q + 32u + r]."""
    rows = arr.shape[0]
    kt = IN // P
    a = arr.reshape(rows, 4, kt, 32)                # [o, q, u, r]
    a = a.transpose(1, 3, 2, 0)                     # [q, r, u, o]
    return np.ascontiguousarray(a.reshape(P, kt, rows))


def prep_inputs(x, qweight, scale, bias):
    """Host-side shard prep. Returns in_maps for run_bass_kernel_spmd."""
    import ml_dtypes
    BF = ml_dtypes.bfloat16
    x = np.asarray(x)
    qw = np.asarray(qweight).reshape(OUT, NG, G)
    sc = np.asarray(scale, dtype=np.float32)
    b = np.asarray(bias, dtype=np.float32)

    # Pre-transposed K-permuted bf16 activations:
    # xt[(m p), (u tc)] = x[128m + tc, 1024*(p//32) + 32u + (p%32)]
    xb = x.reshape(T, IN).astype(BF)
    xt = xb.reshape(T // P, P, 4, IN // P, 32)      # [m, tc, q, u, r]
    xt = xt.transpose(0, 2, 4, 3, 1)                # [m, q, r, u, tc]
    xt = np.ascontiguousarray(xt.reshape(T, IN))

    # Dequantize exactly as the reference does (q / scale, f32), then bf16.
    w = (qw.astype(np.float32) / sc.reshape(OUT, NG, 1)).reshape(OUT, IN)
    w_u16 = w.astype(BF).view(np.uint16)
    bb = b.astype(BF)

    in_maps = []
    for c in range(NCORES):
        sl = slice(c * OUT_SH, (c + 1) * OUT_SH)
        wt = _permute_kt(w_u16[sl]).view(BF)        # [P, kt, OUT_SH]
        # chunk-major: [kt//UCH, P, UCH, OUT_SH], contiguous per chunk
        wtc = np.ascontiguousarray(
            wt.reshape(P, IN // P // UCH, UCH, OUT_SH).transpose(1, 0, 2, 3)
        )
        in_maps.append({
            "xt": xt,
            "wt": wtc,
            "biasb": np.ascontiguousarray(
                np.broadcast_to(bb[sl][None, :], (P, OUT_SH))
            ),
        })
    return in_maps


def run(x, qweight, scale, bias, trace=False):
    nc = _get_nc()
    in_maps = prep_inputs(x, qweight, scale, bias)
    res = run_bass_kernel_spmd(nc, in_maps, core_ids=list(range(NCORES)),
                               trace=trace)
    ys = [np.asarray(res.results[c]["y"]) for c in range(NCORES)]
    out = np.concatenate(ys, axis=1)
    return out.reshape(B, S, OUT).astype(np.float32, copy=False), res


def kernel(x, qweight, scale, bias):
    out, _ = run(x, qweight, scale, bias, trace=False)
    return out


# revision 14
# speedup vs baseline: 1.1909x; 1.0014x over previous
"""Trainium2 Bass kernel for nn_CLinear (group-quantized linear layer).

Computes out = x @ dequant(qweight).T + bias where
  x:       [4, 2048, 4096] f32
  qweight: [11008, 16, 256] int8 (group-quantized, G=256)
  scale:   [11008, 16, 1]   f32  (w = qweight / scale)
  bias:    [11008]          f32
  out:     [4, 2048, 11008] f32

Sharding: column-parallel (tensor-parallel over out_features) across 8
NeuronCores.  11008 = 8 * 1376 exactly, so every core owns a contiguous
1376-column output shard (the matmul free dim needs no 128-alignment).

Per-core kernel structure:
  - x is pre-transposed on the host into K-permuted bf16 lhsT tiles
    xt[(m p), (u tc)] = x[128m + tc, 1024*(p//32) + 32u + (p%32)]: one
    contiguous full-partition 1 MB DMA per token tile, no on-chip
    transpose work.
  - The weight shard is host-dequantized to bf16 in the same K-permuted
    layout and stored chunk-major in DRAM (32 fully contiguous 352 KB
    single-k-tile chunks) so the weight stream reads HBM sequentially.
    Chunks ride the three DGE queues (scalar/gpsimd/sync).
  - Warm-up: ~10 us of throwaway matmuls keep the PE busy from t~2us so
    the HAM clock gate is at 8/8 before the first real matmul, then the
    first TWO token tiles are computed chunk-major (m0/m1 interleaved per
    weight chunk, all 6 PSUM banks accumulating) so the PE does ~37 us of
    useful work while the weight stream lands.
  - Steady state: per token tile, 3 out-blocks x 32 accumulating bf16
    matmuls into PSUM f32; DVE adds bias during PSUM->SBUF evict; the f32
    result DMAs out on the scalar queue.
"""

import numpy as np

import concourse.bass as bass
import concourse.mybir as mybir
import concourse.tile as tile
from concourse import bacc
from concourse.bass_utils import run_bass_kernel_spmd

P = 128
B, S, IN, OUT, G = 4, 2048, 4096, 11008, 256
NCORES = 8
T = B * S                      # 8192 tokens
OUT_SH = OUT // NCORES         # 1376 out features per core (exact)
NG = IN // G                   # 16 quant groups per row
F32 = mybir.dt.float32
BF16 = mybir.dt.bfloat16

UCH = 1                        # k-tiles per weight chunk
NWARM = 64                     # PE warm-up dummy matmuls (~8 us bridge
                               # from engine start to the first weight chunk)
WARM_TILES = 2                 # token tiles computed chunk-major at start


def _n_blocks(out_sh, nmax=512):
    blocks = []
    o = 0
    while o < out_sh:
        sz = min(nmax, out_sh - o)
        blocks.append((o, sz))
        o += sz
    return blocks


def emit_kernel(tc, nc, xt_d, wt_d, bb_d, y_d, t_dim, in_dim, out_sh):
    """Emit the per-core kernel IR.

    xt_d: [t_dim, in_dim]           bf16  (K-permuted pre-transposed x)
    wt_d: [kt//UCH, P, UCH, out_sh] bf16  (chunk-major K-permuted weights)
    bb_d: [P, out_sh]               bf16  (bias broadcast across partitions)
    y_d:  [t_dim, out_sh]           f32   (output shard)
    """
    kt = in_dim // P           # k-tiles (u index)
    mt = t_dim // P            # token tiles
    nchunk = kt // UCH
    nblk = _n_blocks(out_sh)

    from contextlib import ExitStack
    ctx = ExitStack()
    const = ctx.enter_context(tc.tile_pool(name="const", bufs=1))
    wtp = ctx.enter_context(tc.tile_pool(name="wt", bufs=1))
    ytp = ctx.enter_context(tc.tile_pool(name="yt", bufs=5))
    outp = ctx.enter_context(tc.tile_pool(name="out", bufs=6))
    psp = ctx.enter_context(tc.tile_pool(name="psum", bufs=2, space="PSUM"))
    wup = ctx.enter_context(tc.tile_pool(name="wup", bufs=1, space="PSUM"))

    # Priority bands (lower = scheduled earlier among ready work):
    PRI_WARM = 1100000  # dummy matmul warm-up, ahead of everything
    PRI_X0 = 1000000    # first two x tiles: needed before any matmul
    PRI_WT = 900000     # weight chunk stream the warm-up tiles pace
    PRI_X = 500000      # steady-state x prefetch: behind the weight
                        # stream, ahead of normal work (evicts/stores)

    # --- PE warm-up: keep the PE busy from ~2us so the HAM clock gate
    # reaches 8/8 before the first real matmul (and never re-throttles).
    with tc.high_priority(offset=PRI_WARM):
        dummy = const.tile([P, 256], BF16)
        nc.vector.memzero(dummy[:])
        wps = wup.tile([P, 256], F32)
        for _ in range(NWARM):
            nc.tensor.matmul(wps[:], dummy[:, :P], dummy[:],
                             start=True, stop=True)

    def load_yt(m, pri, eng=nc.sync, halves=False):
        with tc.high_priority(offset=pri):
            yt = ytp.tile([P, kt, P], BF16, name="yt")
            row = xt_d[m * P:(m + 1) * P, :]
            if halves:   # first half only; caller DMAs the rest later
                eng.dma_start(
                    yt[:, :kt // 2, :].rearrange("p u tc -> p (u tc)"),
                    row[:, :kt // 2 * P],
                )
            else:
                eng.dma_start(yt.rearrange("p u tc -> p (u tc)"), row)
            return yt

    def load_yt_rest(m, yt, eng):
        eng.dma_start(
            yt[:, kt // 2:, :].rearrange("p u tc -> p (u tc)"),
            xt_d[m * P:(m + 1) * P, kt // 2 * P:],
        )

    # First halves of the warm-up x tiles (all the warm-up needs until
    # k-tile 16) load in parallel on sync/gpsimd ahead of the weight
    # stream; the second halves ride mid-stream below.
    yts = {0: load_yt(0, PRI_X0, eng=nc.sync, halves=True),
           1: load_yt(1, PRI_X0, eng=nc.gpsimd, halves=True)}

    # --- Weight chunk stream.  Byte-balanced static queue assignment:
    # scalar starts empty so it carries chunk 0 (split into out-blocks so
    # the very first matmul only waits for a 128 KB slice) and a slightly
    # larger share; sync/gpsimd start busy with yt0a/yt1a so their chunks
    # skew later.  The warm-up tiles' second halves and the bias slot in
    # before k-tile 16's chunk.
    chunk_q = {}
    for g in (0, 2, 4, 6, 9, 12, 15, 18, 21, 24, 27, 30):
        chunk_q[g] = nc.scalar
    for g in (1, 3, 5, 7, 11, 14, 17, 20, 23, 26, 29):
        chunk_q[g] = nc.gpsimd
    for g in (8, 10, 13, 16, 19, 22, 25, 28, 31):
        chunk_q[g] = nc.sync

    wts = []
    with tc.high_priority(offset=PRI_WT):
        for g in range(nchunk):
            if g == kt // 2:
                load_yt_rest(0, yts[0], nc.sync)
                load_yt_rest(1, yts[1], nc.gpsimd)
                biasb = const.tile([P, out_sh], BF16)
                nc.sync.dma_start(biasb[:], bb_d[:, :])
            wtt = wtp.tile([P, UCH, out_sh], BF16, name=f"wt{g}")
            if g == 0:
                for n0, sz in nblk:
                    nc.scalar.dma_start(wtt[:, :, n0:n0 + sz],
                                        wt_d[g][:, :, n0:n0 + sz])
            else:
                chunk_q[g].dma_start(wtt[:], wt_d[g])
            wts.append(wtt)

    for m in range(2, min(5, mt)):
        yts[m] = load_yt(m, PRI_X)

    pending = []   # psums awaiting evict, evicted one tile late so the
                   # DVE never blocks the PE's critical path.

    def evict(m, nb, n0, sz, ps):
        t0 = m * P
        ot = outp.tile([P, 512], F32, name="ot")
        nc.vector.tensor_tensor(
            ot[:, :sz], ps, biasb[:, n0:n0 + sz], mybir.AluOpType.add
        )
        nc.scalar.dma_start(y_d[t0:t0 + P, n0:n0 + sz], ot[:, :sz])

    # --- Warm-up tiles m=0..WARM_TILES-1, chunk-major: both tiles'
    # accumulation groups stay open across the whole k loop (6 PSUM
    # banks) and consume each weight chunk as it lands.
    wt_ps = {}
    for m in range(WARM_TILES):
        for nb, (n0, sz) in enumerate(nblk):
            wt_ps[(m, nb)] = psp.tile([P, 512], F32, name=f"ps{nb}")[:, :sz]
    for g in range(nchunk):
        for m in range(WARM_TILES):
            for uu in range(UCH):
                u = g * UCH + uu
                for nb, (n0, sz) in enumerate(nblk):
                    nc.tensor.matmul(
                        wt_ps[(m, nb)],
                        yts[m][:, u, :],
                        wts[g][:, uu, n0:n0 + sz],
                        start=(u == 0),
                        stop=(u == kt - 1),
                        skip_group_check=True,
                    )
    for m in range(WARM_TILES):
        for nb, (n0, sz) in enumerate(nblk):
            pending.append((m, nb, n0, sz, wt_ps[(m, nb)]))
        yts.pop(m)

    # --- Steady state.
    for m in range(WARM_TILES, mt):
        if m + 3 < mt and (m + 3) not in yts:
            yts[m + 3] = load_yt(m + 3, PRI_X)
        for args in pending:
            evict(*args)
        pending = []
        ytf = yts.pop(m)
        for nb, (n0, sz) in enumerate(nblk):
            ps = psp.tile([P, 512], F32, name=f"ps{nb}")[:, :sz]
            for u in range(kt):
                nc.tensor.matmul(
                    ps,
                    ytf[:, u, :],  # [P, 128] contiguous: tokens t0..t0+127
                    wts[u // UCH][:, u % UCH, n0:n0 + sz],
                    start=(u == 0),
                    stop=(u == kt - 1),
                )
            pending.append((m, nb, n0, sz, ps))
    for args in pending:
        evict(*args)

    ctx.close()


def build_nc(t_dim=T, in_dim=IN, out_sh=OUT_SH, debug=False):
    kt = in_dim // P
    nc = bacc.Bacc(
        "TRN2",
        target_bir_lowering=False,
        debug=debug,
        num_devices=NCORES,
        enable_asserts=debug,
    )
    xt_d = nc.dram_tensor("xt", [t_dim, in_dim], BF16, kind="ExternalInput").ap()
    wt_d = nc.dram_tensor("wt", [kt // UCH, P, UCH, out_sh], BF16,
                          kind="ExternalInput").ap()
    bb_d = nc.dram_tensor("biasb", [P, out_sh], BF16, kind="ExternalInput").ap()
    y_d = nc.dram_tensor("y", [t_dim, out_sh], F32, kind="ExternalOutput").ap()
    with tile.TileContext(nc) as tc:
        emit_kernel(tc, nc, xt_d, wt_d, bb_d, y_d, t_dim, in_dim, out_sh)
    nc.compile()
    return nc


_NC_CACHE = {}


def _get_nc():
    if "nc" not in _NC_CACHE:
        _NC_CACHE["nc"] = build_nc()
    return _NC_CACHE["nc"]


def _permute_kt(arr):
    """[rows, IN] -> K-permuted transposed [P, kt, rows]:
    out[32q+r, u, o] = arr[o, 1024q + 32u + r]."""
    rows = arr.shape[0]
    kt = IN // P
    a = arr.reshape(rows, 4, kt, 32)                # [o, q, u, r]
    a = a.transpose(1, 3, 2, 0)                     # [q, r, u, o]
    return np.ascontiguousarray(a.reshape(P, kt, rows))


def prep_inputs(x, qweight, scale, bias):
    """Host-side shard prep. Returns in_maps for run_bass_kernel_spmd."""
    import ml_dtypes
    BF = ml_dtypes.bfloat16
    x = np.asarray(x)
    qw = np.asarray(qweight).reshape(OUT, NG, G)
    sc = np.asarray(scale, dtype=np.float32)
    b = np.asarray(bias, dtype=np.float32)

    # Pre-transposed K-permuted bf16 activations:
    # xt[(m p), (u tc)] = x[128m + tc, 1024*(p//32) + 32u + (p%32)]
    xb = x.reshape(T, IN).astype(BF)
    xt = xb.reshape(T // P, P, 4, IN // P, 32)      # [m, tc, q, u, r]
    xt = xt.transpose(0, 2, 4, 3, 1)                # [m, q, r, u, tc]
    xt = np.ascontiguousarray(xt.reshape(T, IN))

    # Dequantize exactly as the reference does (q / scale, f32), then bf16.
    w = (qw.astype(np.float32) / sc.reshape(OUT, NG, 1)).reshape(OUT, IN)
    w_u16 = w.astype(BF).view(np.uint16)
    bb = b.astype(BF)

    in_maps = []
    for c in range(NCORES):
        sl = slice(c * OUT_SH, (c + 1) * OUT_SH)
        wt = _permute_kt(w_u16[sl]).view(BF)        # [P, kt, OUT_SH]
        # chunk-major: [kt//UCH, P, UCH, OUT_SH], contiguous per chunk
        wtc = np.ascontiguousarray(
            wt.reshape(P, IN // P // UCH, UCH, OUT_SH).transpose(1, 0, 2, 3)
        )
        in_maps.append({
            "xt": xt,
            "wt": wtc,
            "biasb": np.ascontiguousarray(
                np.broadcast_to(bb[sl][None, :], (P, OUT_SH))
            ),
        })
    return in_maps


def run(x, qweight, scale, bias, trace=False):
    nc = _get_nc()
    in_maps = prep_inputs(x, qweight, scale, bias)
    res = run_bass_kernel_spmd(nc, in_maps, core_ids=list(range(NCORES)),
                               trace=trace)
    ys = [np.asarray(res.results[c]["y"]) for c in range(NCORES)]
    out = np.concatenate(ys, axis=1)
    return out.reshape(B, S, OUT).astype(np.float32, copy=False), res


def kernel(x, qweight, scale, bias):
    out, _ = run(x, qweight, scale, bias, trace=False)
    return out


# revision 18
# speedup vs baseline: 1.1920x; 1.0009x over previous
"""Trainium2 Bass kernel for nn_CLinear (group-quantized linear layer).

Computes out = x @ dequant(qweight).T + bias where
  x:       [4, 2048, 4096] f32
  qweight: [11008, 16, 256] int8 (group-quantized, G=256)
  scale:   [11008, 16, 1]   f32  (w = qweight / scale)
  bias:    [11008]          f32
  out:     [4, 2048, 11008] f32

Sharding: column-parallel (tensor-parallel over out_features) across 8
NeuronCores.  11008 = 8 * 1376 exactly, so every core owns a contiguous
1376-column output shard (the matmul free dim needs no 128-alignment).

Per-core kernel structure:
  - x is pre-transposed on the host into K-permuted bf16 lhsT tiles
    xt[(m p), (u tc)] = x[128m + tc, 1024*(p//32) + 32u + (p%32)]: one
    contiguous full-partition 1 MB DMA per token tile, no on-chip
    transpose work.
  - The weight shard is host-dequantized to bf16 in the same K-permuted
    layout and stored chunk-major in DRAM (32 fully contiguous 352 KB
    single-k-tile chunks) so the weight stream reads HBM sequentially.
    Chunks ride the three DGE queues (scalar/gpsimd/sync).
  - Warm-up: ~10 us of throwaway matmuls keep the PE busy from t~2us so
    the HAM clock gate is at 8/8 before the first real matmul, then the
    first TWO token tiles are computed chunk-major (m0/m1 interleaved per
    weight chunk, all 6 PSUM banks accumulating) so the PE does ~37 us of
    useful work while the weight stream lands.
  - Steady state: per token tile, 3 out-blocks x 32 accumulating bf16
    matmuls into PSUM f32; DVE adds bias during PSUM->SBUF evict; the f32
    result DMAs out on the scalar queue.
"""

import numpy as np

import concourse.bass as bass
import concourse.mybir as mybir
import concourse.tile as tile
from concourse import bacc
from concourse.bass_utils import run_bass_kernel_spmd

P = 128
B, S, IN, OUT, G = 4, 2048, 4096, 11008, 256
NCORES = 8
T = B * S                      # 8192 tokens
OUT_SH = OUT // NCORES         # 1376 out features per core (exact)
NG = IN // G                   # 16 quant groups per row
F32 = mybir.dt.float32
BF16 = mybir.dt.bfloat16

UCH = 1                        # k-tiles per weight chunk
NWARM = 48                     # PE warm-up dummy matmuls (~6.5 us bridge
                               # from engine start to the first weight chunk)
WARM_TILES = 2                 # token tiles computed chunk-major at start


def _n_blocks(out_sh, nmax=512):
    blocks = []
    o = 0
    while o < out_sh:
        sz = min(nmax, out_sh - o)
        blocks.append((o, sz))
        o += sz
    return blocks


def emit_kernel(tc, nc, xt_d, wt_d, bb_d, y_d, t_dim, in_dim, out_sh):
    """Emit the per-core kernel IR.

    xt_d: [t_dim, in_dim]           bf16  (K-permuted pre-transposed x)
    wt_d: [kt//UCH, P, UCH, out_sh] bf16  (chunk-major K-permuted weights)
    bb_d: [P, out_sh]               bf16  (bias broadcast across partitions)
    y_d:  [t_dim, out_sh]           f32   (output shard)
    """
    kt = in_dim // P           # k-tiles (u index)
    mt = t_dim // P            # token tiles
    nchunk = kt // UCH
    nblk = _n_blocks(out_sh)

    from contextlib import ExitStack
    ctx = ExitStack()
    const = ctx.enter_context(tc.tile_pool(name="const", bufs=1))
    wtp = ctx.enter_context(tc.tile_pool(name="wt", bufs=1))
    ytp = ctx.enter_context(tc.tile_pool(name="yt", bufs=5))
    outp = ctx.enter_context(tc.tile_pool(name="out", bufs=6))
    psp = ctx.enter_context(tc.tile_pool(name="psum", bufs=2, space="PSUM"))
    wup = ctx.enter_context(tc.tile_pool(name="wup", bufs=1, space="PSUM"))

    # Priority bands (lower = scheduled earlier among ready work):
    PRI_WARM = 1100000  # dummy matmul warm-up, ahead of everything
    PRI_X0 = 1000000    # first two x tiles: needed before any matmul
    PRI_WT = 900000     # weight chunk stream the warm-up tiles pace
    PRI_X = 500000      # steady-state x prefetch: behind the weight
                        # stream, ahead of normal work (evicts/stores)

    # --- PE warm-up: keep the PE busy from ~2us so the HAM clock gate
    # reaches 8/8 before the first real matmul (and never re-throttles).
    with tc.high_priority(offset=PRI_WARM):
        # 4 KB primer DMAs warm each DGE ring before the critical stream.
        for i, eng in enumerate((nc.sync, nc.scalar, nc.gpsimd)):
            prim = const.tile([P, 16], BF16, name=f"prim{i}")
            eng.dma_start(prim[:], bb_d[:, :16])
        dummy = const.tile([P, 256], BF16)
        nc.vector.memzero(dummy[:])
        wps = wup.tile([P, 256], F32)
        for _ in range(NWARM):
            nc.tensor.matmul(wps[:], dummy[:, :P], dummy[:],
                             start=True, stop=True)

    def load_yt(m, pri, eng=nc.sync):
        with tc.high_priority(offset=pri):
            yt = ytp.tile([P, kt, P], BF16, name="yt")
            eng.dma_start(
                yt.rearrange("p u tc -> p (u tc)"),
                xt_d[m * P:(m + 1) * P, :],
            )
            return yt

    def load_yt_piece(m, yt, u0, u1, eng):
        eng.dma_start(
            yt[:, u0:u1, :].rearrange("p u tc -> p (u tc)"),
            xt_d[m * P:(m + 1) * P, u0 * P:u1 * P],
        )

    # The warm-up x tiles load in pieces sized to the chunk-major sweep's
    # needs: k-tiles 0-7 first (256 KB critical pieces), 8-15 and 16-31
    # slotted mid-stream below, so the ramping DMA queues carry as little
    # as possible ahead of the early weight chunks.
    with tc.high_priority(offset=PRI_X0):
        yt0 = ytp.tile([P, kt, P], BF16, name="yt")
        load_yt_piece(0, yt0, 0, kt // 4, nc.sync)
        yt1 = ytp.tile([P, kt, P], BF16, name="yt")
        load_yt_piece(1, yt1, 0, kt // 4, nc.gpsimd)
    yts = {0: yt0, 1: yt1}

    # --- Weight chunk stream.  Byte-balanced static queue assignment:
    # scalar starts empty so it carries chunk 0 (split into out-blocks so
    # the very first matmul only waits for a 128 KB slice) and a slightly
    # larger share; sync/gpsimd start busy with yt0a/yt1a so their chunks
    # skew later.  The warm-up tiles' second halves and the bias slot in
    # before k-tile 16's chunk.
    chunk_q = {}
    for g in (0, 2, 4, 6, 9, 12, 15, 18, 21, 24, 27, 30):
        chunk_q[g] = nc.scalar
    for g in (1, 3, 5, 7, 11, 14, 17, 20, 23, 26, 29):
        chunk_q[g] = nc.gpsimd
    for g in (8, 10, 13, 16, 19, 22, 25, 28, 31):
        chunk_q[g] = nc.sync

    wts = []
    with tc.high_priority(offset=PRI_WT):
        for g in range(nchunk):
            if g == 6:
                load_yt_piece(0, yt0, kt // 4, kt // 2, nc.sync)
                load_yt_piece(1, yt1, kt // 4, kt // 2, nc.gpsimd)
            if g == kt // 2:
                load_yt_piece(0, yt0, kt // 2, kt, nc.sync)
                load_yt_piece(1, yt1, kt // 2, kt, nc.gpsimd)
                biasb = const.tile([P, out_sh], BF16)
                nc.sync.dma_start(biasb[:], bb_d[:, :])
            wtt = wtp.tile([P, UCH, out_sh], BF16, name=f"wt{g}")
            if g == 0:
                for n0, sz in nblk:
                    nc.scalar.dma_start(wtt[:, :, n0:n0 + sz],
                                        wt_d[g][:, :, n0:n0 + sz])
            else:
                chunk_q[g].dma_start(wtt[:], wt_d[g])
            wts.append(wtt)

    for m in range(2, min(5, mt)):
        yts[m] = load_yt(m, PRI_X)

    pending = []   # psums awaiting evict, evicted one tile late so the
                   # DVE never blocks the PE's critical path.

    def evict(m, nb, n0, sz, ps):
        t0 = m * P
        ot = outp.tile([P, 512], F32, name="ot")
        nc.vector.tensor_tensor(
            ot[:, :sz], ps, biasb[:, n0:n0 + sz], mybir.AluOpType.add
        )
        nc.scalar.dma_start(y_d[t0:t0 + P, n0:n0 + sz], ot[:, :sz])

    # --- Warm-up tiles m=0..WARM_TILES-1, chunk-major: both tiles'
    # accumulation groups stay open across the whole k loop (6 PSUM
    # banks) and consume each weight chunk as it lands.
    wt_ps = {}
    for m in range(WARM_TILES):
        for nb, (n0, sz) in enumerate(nblk):
            wt_ps[(m, nb)] = psp.tile([P, 512], F32, name=f"ps{nb}")[:, :sz]
    for g in range(nchunk):
        for m in range(WARM_TILES):
            for uu in range(UCH):
                u = g * UCH + uu
                for nb, (n0, sz) in enumerate(nblk):
                    nc.tensor.matmul(
                        wt_ps[(m, nb)],
                        yts[m][:, u, :],
                        wts[g][:, uu, n0:n0 + sz],
                        start=(u == 0),
                        stop=(u == kt - 1),
                        skip_group_check=True,
                    )
    for m in range(WARM_TILES):
        for nb, (n0, sz) in enumerate(nblk):
            pending.append((m, nb, n0, sz, wt_ps[(m, nb)]))
        yts.pop(m)

    # --- Steady state.
    for m in range(WARM_TILES, mt):
        if m + 3 < mt and (m + 3) not in yts:
            yts[m + 3] = load_yt(m + 3, PRI_X)
        for args in pending:
            evict(*args)
        pending = []
        ytf = yts.pop(m)
        for nb, (n0, sz) in enumerate(nblk):
            ps = psp.tile([P, 512], F32, name=f"ps{nb}")[:, :sz]
            for u in range(kt):
                nc.tensor.matmul(
                    ps,
                    ytf[:, u, :],  # [P, 128] contiguous: tokens t0..t0+127
                    wts[u // UCH][:, u % UCH, n0:n0 + sz],
                    start=(u == 0),
                    stop=(u == kt - 1),
                )
            if m == mt - 1:
                # Final tile: evict each block as soon as its accumulation
                # closes so the tail drain overlaps the last matmuls.
                evict(m, nb, n0, sz, ps)
            else:
                pending.append((m, nb, n0, sz, ps))
    for args in pending:
        evict(*args)

    ctx.close()


def build_nc(t_dim=T, in_dim=IN, out_sh=OUT_SH, debug=False):
    kt = in_dim // P
    nc = bacc.Bacc(
        "TRN2",
        target_bir_lowering=False,
        debug=debug,
        num_devices=NCORES,
        enable_asserts=debug,
    )
    xt_d = nc.dram_tensor("xt", [t_dim, in_dim], BF16, kind="ExternalInput").ap()
    wt_d = nc.dram_tensor("wt", [kt // UCH, P, UCH, out_sh], BF16,
                          kind="ExternalInput").ap()
    bb_d = nc.dram_tensor("biasb", [P, out_sh], BF16, kind="ExternalInput").ap()
    y_d = nc.dram_tensor("y", [t_dim, out_sh], F32, kind="ExternalOutput").ap()
    with tile.TileContext(nc) as tc:
        emit_kernel(tc, nc, xt_d, wt_d, bb_d, y_d, t_dim, in_dim, out_sh)
    nc.compile()
    return nc


_NC_CACHE = {}


def _get_nc():
    if "nc" not in _NC_CACHE:
        _NC_CACHE["nc"] = build_nc()
    return _NC_CACHE["nc"]


def _permute_kt(arr):
    """[rows, IN] -> K-permuted transposed [P, kt, rows]:
    out[32q+r, u, o] = arr[o, 1024q + 32u + r]."""
    rows = arr.shape[0]
    kt = IN // P
    a = arr.reshape(rows, 4, kt, 32)                # [o, q, u, r]
    a = a.transpose(1, 3, 2, 0)                     # [q, r, u, o]
    return np.ascontiguousarray(a.reshape(P, kt, rows))


def prep_inputs(x, qweight, scale, bias):
    """Host-side shard prep. Returns in_maps for run_bass_kernel_spmd."""
    import ml_dtypes
    BF = ml_dtypes.bfloat16
    x = np.asarray(x)
    qw = np.asarray(qweight).reshape(OUT, NG, G)
    sc = np.asarray(scale, dtype=np.float32)
    b = np.asarray(bias, dtype=np.float32)

    # Pre-transposed K-permuted bf16 activations:
    # xt[(m p), (u tc)] = x[128m + tc, 1024*(p//32) + 32u + (p%32)]
    xb = x.reshape(T, IN).astype(BF)
    xt = xb.reshape(T // P, P, 4, IN // P, 32)      # [m, tc, q, u, r]
    xt = xt.transpose(0, 2, 4, 3, 1)                # [m, q, r, u, tc]
    xt = np.ascontiguousarray(xt.reshape(T, IN))

    # Dequantize exactly as the reference does (q / scale, f32), then bf16.
    w = (qw.astype(np.float32) / sc.reshape(OUT, NG, 1)).reshape(OUT, IN)
    w_u16 = w.astype(BF).view(np.uint16)
    bb = b.astype(BF)

    in_maps = []
    for c in range(NCORES):
        sl = slice(c * OUT_SH, (c + 1) * OUT_SH)
        wt = _permute_kt(w_u16[sl]).view(BF)        # [P, kt, OUT_SH]
        # chunk-major: [kt//UCH, P, UCH, OUT_SH], contiguous per chunk
        wtc = np.ascontiguousarray(
            wt.reshape(P, IN // P // UCH, UCH, OUT_SH).transpose(1, 0, 2, 3)
        )
        in_maps.append({
            "xt": xt,
            "wt": wtc,
            "biasb": np.ascontiguousarray(
                np.broadcast_to(bb[sl][None, :], (P, OUT_SH))
            ),
        })
    return in_maps


def run(x, qweight, scale, bias, trace=False):
    nc = _get_nc()
    in_maps = prep_inputs(x, qweight, scale, bias)
    res = run_bass_kernel_spmd(nc, in_maps, core_ids=list(range(NCORES)),
                               trace=trace)
    ys = [np.asarray(res.results[c]["y"]) for c in range(NCORES)]
    out = np.concatenate(ys, axis=1)
    return out.reshape(B, S, OUT).astype(np.float32, copy=False), res


def kernel(x, qweight, scale, bias):
    out, _ = run(x, qweight, scale, bias, trace=False)
    return out
